# revision 10
# baseline (speedup 1.0000x reference)
"""ConstituencyAwareAttention Trainium2 kernel.

Strategy: pure data parallelism -- B=8 batch elements across 8 NeuronCores,
one full attention problem per core (S=1024, H=1024, nh=16, hd=64). No
collectives.

Per-core pipeline (fp16 matmul inputs, fp32 accumulation):

  * Inputs arrive pre-transposed AND pre-cast to fp16 from the host (pure
    layout/precision staging, the kernel computes in fp16 regardless), which
    halves the startup HBM load and removes every on-device cast.
  * Input DMAs are split into need-ordered pieces on three queues; X^T is
    loaded twice (once as s-quarters feeding the V projection's stationary
    operands, once as s-halves feeding the Q/K projections' moving operands)
    so the V projection starts as soon as ~1.5 MB has landed. Dummy warm-up
    matmuls keep the PE's DVFS ramp hot during the initial DMA wait.
  * All PSUM pools are double-buffered (proj x2, score x2, ctx x2 = 8 banks)
    so back-to-back accumulation groups never wait on drains.
  * Emission is software-pipelined: per pair, score matmuls (2 co-issued
    64-row matmuls per kt via tile_position) are emitted in batches of two
    kt-slots between independent filler groups (AV of the previous pair, Q/K
    projection of the next pair, deferred V-projection groups). Batching
    by two halves the number of sc<->full-K boundaries, each of which costs
    ~100ns because a tile_position LDWEIGHTS cannot overlap a full-128-row
    matmul still in flight. The score-PSUM rotation (bufs=2) tolerates
    exactly two outstanding kt slots before gating on the scalar exp.
  * The constituency fixup (undo the -0.5 penalty inside same-constituent
    [64,64] blocks) is one vector tensor_mul per (kt, half) against a
    constant [128,128] mask tile; exp itself applies the -0.5 as bias.
  * ctx is produced transposed ([65, q] per head: 64 V dims + the softmax
    denominator from a ones-column in V_aug) and DMA'd out unnormalized as
    fp16, one DMA per head. The host divides by the denominator row and
    transposes back -- no PE transposes, reciprocals or normalize multiplies
    on device.
"""

import math
import sys

if "/opt/trn_rl_repo" not in sys.path:
    sys.path.insert(0, "/opt/trn_rl_repo")

import numpy as np

import concourse.bacc as bacc
import concourse.tile as tile
from concourse import mybir
from concourse.bass_utils import run_bass_kernel_spmd

F16 = mybir.dt.float16
F32 = mybir.dt.float32

B, S, H = 8, 1024, 1024
NH, HD = 16, 64
OA = HD + 1   # per-head output rows: 64 ctx dims + denominator
P = 128
SO = S // P   # 8 S-chunks
KO = H // P   # 8 contraction chunks
PEN = 0.5
FIX = float(math.exp(PEN))
SCALE = 1.0 / math.sqrt(HD)
N_WARM = 40   # PE warm-up matmuls during the initial DMA wait

_programs = {}


def _build_program(with_bv: bool):
    nc = bacc.Bacc("TRN2", target_bir_lowering=False, debug=False)

    xt = nc.dram_tensor("xt", [H, S], F16, kind="ExternalInput").ap()
    wq = nc.dram_tensor("wq", [H, H], F16, kind="ExternalInput").ap()
    wk = nc.dram_tensor("wk", [H, H], F16, kind="ExternalInput").ap()
    wv = nc.dram_tensor("wv", [H, H], F16, kind="ExternalInput").ap()
    bq = nc.dram_tensor("bq", [H], F32, kind="ExternalInput").ap()
    bk = nc.dram_tensor("bk", [H], F32, kind="ExternalInput").ap()
    bv = nc.dram_tensor("bv", [H], F32, kind="ExternalInput").ap()
    # transposed, unnormalized output: per head 65 rows (64 ctx + denom)
    out = nc.dram_tensor("out", [NH * OA, S], F16, kind="ExternalOutput").ap()

    Exp = mybir.ActivationFunctionType.Exp

    with tile.TileContext(nc) as tc:
        with tc.tile_pool(name="persist", bufs=1) as persist:
            XTh = [persist.tile([P, KO, 512], F16, name=f"XTh{j}") for j in range(2)]
            WVh = [persist.tile([P, KO, 512], F16, name=f"WV{j}") for j in range(2)]
            WQp = [persist.tile([P, KO, 512], F16, name=f"WQ{j}") for j in range(2)]
            WKp = [persist.tile([P, KO, 512], F16, name=f"WK{j}") for j in range(2)]
            QT = [persist.tile([P, S], F16, name=f"QT{m}") for m in range(NH // 2)]
            KT = [persist.tile([P, S], F16, name=f"KT{m}") for m in range(NH // 2)]
            VA = persist.tile([P, SO, NH * OA], F16, name="VA")
            fixm = persist.tile([P, P], F16, name="fixm")
            scratch = persist.tile([P, 512], F16, name="scratch")
            nbias = persist.tile([P, 1], F32, name="nbias")
            bq_s = persist.tile([P, KO], F32, name="bq_s")
            bk_s = persist.tile([P, KO], F32, name="bk_s")

            nc.vector.memset(nbias[:], -PEN)
            # constituency fixup mask: e^PEN on the two same-block squares
            nc.vector.memset(fixm[:], 1.0)
            nc.vector.memset(fixm[0:64, 0:64], FIX)
            nc.vector.memset(fixm[64:128, 64:128], FIX)
            nc.vector.memset(scratch[:], 0.0)
            # ones-columns of V_aug (drains below only write the V parts)
            va_h = VA[:].rearrange("p s (h c) -> p s h c", c=OA)
            nc.vector.memset(va_h[:, :, :, HD : HD + 1], 1.0)

            # ---- input DMAs: three queues, need-ordered ----
            xt_r = xt.rearrange("(ho hp) s -> hp ho s", hp=P)
            wv_r = wv.rearrange("(kp_o kp) n -> kp kp_o n", kp=P)
            wq_r = wq.rearrange("(kp_o kp) n -> kp kp_o n", kp=P)
            wk_r = wk.rearrange("(kp_o kp) n -> kp kp_o n", kp=P)

            nc.sync.dma_start(out=WVh[0][:], in_=wv_r[:, :, 0:512])
            nc.gpsimd.dma_start(out=XTh[0][:], in_=xt_r[:, :, 0:512])
            nc.sync.dma_start(out=WVh[1][:], in_=wv_r[:, :, 512:1024])
            nc.gpsimd.dma_start(out=XTh[1][:], in_=xt_r[:, :, 512:1024])
            nc.scalar.dma_start(out=bq_s[:], in_=bq.rearrange("(o p) -> p o", p=P))
            nc.vector.tensor_scalar_mul(bq_s[:], bq_s[:], SCALE)
            nc.scalar.dma_start(out=bk_s[:], in_=bk.rearrange("(o p) -> p o", p=P))
            for j in range(2):
                nc.sync.dma_start(
                    out=WQp[j][:], in_=wq_r[:, :, j * 512 : (j + 1) * 512]
                )
                nc.gpsimd.dma_start(
                    out=WKp[j][:], in_=wk_r[:, :, j * 512 : (j + 1) * 512]
                )

            with (
                tc.tile_pool(name="attn", bufs=1) as attn,
                tc.tile_pool(name="proj_ps", bufs=2, space="PSUM") as proj_ps,
                tc.tile_pool(name="score_ps", bufs=2, space="PSUM") as score_ps,
                tc.tile_pool(name="ctx_ps", bufs=2, space="PSUM") as ctx_ps,
            ):
                # PE warm-up: keep the DVFS ramp hot while inputs stream in.
                for _ in range(N_WARM):
                    ps = proj_ps.tile([P, 512], F32, name="ps", tag="proj")
                    nc.tensor.matmul(
                        ps[:], fixm[:], scratch[:], start=True, stop=True
                    )

                # ---- V projection: [S,h_out] blocks into V_aug ----
                def vproj_group(so, ncol):
                    ps = proj_ps.tile([P, 512], F32, name="ps", tag="proj")
                    sq, soc = divmod(so, 4)
                    for kh in range(KO):
                        nc.tensor.matmul(
                            ps[:],
                            XTh[sq][:, kh, soc * P : (soc + 1) * P],
                            WVh[ncol][:, kh, :],
                            start=(kh == 0),
                            stop=(kh == KO - 1),
                        )
                    va_v = VA[:, so, :].rearrange("p (h c) -> p h c", c=OA)
                    nc.vector.tensor_copy(
                        va_v[:, ncol * 8 : (ncol + 1) * 8, 0:HD],
                        ps[:].rearrange("p (h c) -> p h c", c=HD),
                    )

                # heads 0-7 upfront; heads 8-15 deferred into iter 0/1 fillers
                # (with_bv needs all of VA early for the bvb add, so no defer)
                for so in range(SO):
                    vproj_group(so, 0)
                deferred_vproj = [
                    (lambda so=so: vproj_group(so, 1)) for so in range(SO)
                ]
                if with_bv:
                    for f in deferred_vproj:
                        f()
                    deferred_vproj = []
                    # out += bv exactly (softmax rows sum to 1), via a
                    # PE-broadcast of bv across partitions added to V_aug.
                    ones1 = persist.tile([1, P], F16, name="ones1")
                    nc.vector.memset(ones1[:], 1.0)
                    bv1 = persist.tile([1, H], F16, name="bv1")
                    bv1_32 = persist.tile([1, H], F32, name="bv1_32")
                    nc.sync.dma_start(out=bv1_32[:], in_=bv[None, :])
                    nc.vector.tensor_copy(bv1[:], bv1_32[:])
                    bvb = persist.tile([P, NH * OA], F16, name="bvb")
                    nc.vector.memset(bvb[:], 0.0)
                    bvb_v = bvb.rearrange("p (h c) -> p h c", c=OA)
                    for ncol in range(2):
                        psb = proj_ps.tile([P, 512], F32, name="psb", tag="proj")
                        nc.tensor.matmul(
                            psb[:], ones1[:], bv1[:, ncol * 512 : (ncol + 1) * 512],
                            start=True, stop=True,
                        )
                        nc.vector.tensor_copy(
                            bvb_v[:, ncol * 8 : (ncol + 1) * 8, 0:HD],
                            psb[:].rearrange("p (h c) -> p h c", c=HD),
                        )
                    for so in range(SO):
                        nc.vector.tensor_add(VA[:, so, :], VA[:, so, :], bvb[:])

                # ---- one Q or K projection accumulation group ----
                def qk_group(m, which, sc):
                    ps = proj_ps.tile([P, 512], F32, name="ps", tag="proj")
                    wsb = (WQp if which == 0 else WKp)[m // 4]
                    col0 = (m % 4) * P
                    for kh in range(KO):
                        nc.tensor.matmul(
                            ps[:],
                            wsb[:, kh, col0 : col0 + P],
                            XTh[sc][:, kh, :],
                            start=(kh == 0),
                            stop=(kh == KO - 1),
                        )
                    if which == 0:
                        nc.vector.tensor_scalar(
                            QT[m][:, sc * 512 : (sc + 1) * 512], ps[:],
                            SCALE, bq_s[:, m : m + 1],
                            mybir.AluOpType.mult, mybir.AluOpType.add,
                        )
                    else:
                        nc.vector.tensor_scalar_add(
                            KT[m][:, sc * 512 : (sc + 1) * 512], ps[:],
                            bk_s[:, m : m + 1],
                        )

                def qk_chunk_fillers(m):
                    return [
                        (lambda w=w, sc=sc: qk_group(m, w, sc))
                        for w in range(2)
                        for sc in range(2)
                    ]

                # prT tiles rotate 3 buffers; 2 allocated per pair
                def alloc_prT():
                    return attn.tile([P, KO, S], F16, name="prT", tag="probsT", bufs=5)

                # ---- scores + exp + fixup for one (pair, kt) ----
                def sc_slot(i, kt, prT):
                    pst = [
                        score_ps.tile([P, S], F32, name="pst", tag="score")
                        for _ in range(2)
                    ]
                    for qc in range(2):
                        for half in range(2):
                            lo = half * 64
                            nc.tensor.matmul(
                                pst[half][:, qc * 512 : (qc + 1) * 512],
                                KT[i][lo : lo + 64, kt * P : (kt + 1) * P],
                                QT[i][lo : lo + 64, qc * 512 : (qc + 1) * 512],
                                start=True,
                                stop=True,
                                tile_position=(lo, 0),
                            )
                    for half in range(2):
                        nc.scalar.activation(
                            prT[half][:, kt, :], pst[half][:], Exp, bias=nbias[:]
                        )
                        # undo the penalty inside same-constituent squares
                        nc.vector.tensor_mul(
                            prT[half][:, kt, kt * P : (kt + 1) * P],
                            prT[half][:, kt, kt * P : (kt + 1) * P],
                            fixm[:],
                        )

                # ---- AV for one (pair, half, qc): ctx^T [65, 512] ----
                def av_group(i, half, qc, prT, sb, split_dma=False):
                    h = 2 * i + half
                    ctxp = ctx_ps.tile([OA, 512], F32, name="ctxp", tag="ctx")
                    for kt in range(KO):
                        nc.tensor.matmul(
                            ctxp[:],
                            VA[:, kt, h * OA : (h + 1) * OA],
                            prT[half][:, kt, qc * 512 : (qc + 1) * 512],
                            start=(kt == 0),
                            stop=(kt == KO - 1),
                        )
                    nc.vector.tensor_copy(sb[:, qc * 512 : (qc + 1) * 512], ctxp[:])
                    if split_dma:
                        nc.gpsimd.dma_start(
                            out=out[h * OA : (h + 1) * OA,
                                    qc * 512 : (qc + 1) * 512],
                            in_=sb[:, qc * 512 : (qc + 1) * 512],
                        )
                    elif qc == 1:
                        nc.gpsimd.dma_start(
                            out=out[h * OA : (h + 1) * OA, :], in_=sb[:]
                        )

                def av_pair_fillers(i, prT, split_dma=False):
                    fills = []
                    for half in range(2):
                        sb = attn.tile([OA, S], F16, name="ctxsb", tag="ctxsb",
                                       bufs=4)
                        fills += [
                            (lambda h=half, qc=qc, pp=prT, s=sb:
                             av_group(i, h, qc, pp, s, split_dma))
                            for qc in range(2)
                        ]
                    return fills

                # ---- pipeline ----
                for f in qk_chunk_fillers(0):
                    f()

                prev_prT = None
                prT = [alloc_prT(), alloc_prT()]
                for i in range(NH // 2):
                    fillers = []
                    if prev_prT is not None:
                        fillers += av_pair_fillers(i - 1, prev_prT)
                    if i + 1 < NH // 2:
                        fillers += qk_chunk_fillers(i + 1)
                    if deferred_vproj:
                        # iter 0 tops up to 8 fillers; iter 1 takes the rest
                        # (extras are emitted after the batch loop below)
                        take = (max(0, 8 - len(fillers)) if i == 0
                                else len(deferred_vproj))
                        fillers += deferred_vproj[:take]
                        deferred_vproj = deferred_vproj[take:]
                    # interleave: one act-gated score slot, then its share
                    # of independent filler groups to hide the exp latency
                    nf = len(fillers)
                    for kt in range(KO):
                        sc_slot(i, kt, prT)
                        for fj in range(kt * nf // KO, (kt + 1) * nf // KO):
                            fillers[fj]()
                    prev_prT = prT
                    if i + 1 < NH // 2:
                        prT = [alloc_prT(), alloc_prT()]
                for f in av_pair_fillers(NH // 2 - 1, prev_prT, split_dma=True):
                    f()

    nc.compile()
    return nc


def _get_program(with_bv: bool):
    key = with_bv
    if key not in _programs:
        _programs[key] = _build_program(with_bv)
    return _programs[key]


def _in_maps(hidden_states, Wq, bq, Wk, bk, Wv, bv):
    wq = np.ascontiguousarray(Wq, np.float16)
    wk = np.ascontiguousarray(Wk, np.float16)
    wv = np.ascontiguousarray(Wv, np.float16)
    bq = np.ascontiguousarray(bq, np.float32)
    bk = np.ascontiguousarray(bk, np.float32)
    bv = np.ascontiguousarray(bv, np.float32)
    hs16 = np.asarray(hidden_states, np.float16)
    return [
        {
            "xt": np.ascontiguousarray(hs16[b].T),
            "wq": wq, "wk": wk, "wv": wv, "bq": bq, "bk": bk, "bv": bv,
        }
        for b in range(B)
    ]


def _postprocess(outT):
    # outT: [NH*OA, S] fp16, per head 64 unnormalized ctx^T rows + denom row
    r = outT.astype(np.float32).reshape(NH, OA, S)
    ctx = r[:, :HD, :] / r[:, HD : HD + 1, :]          # [NH, HD, S]
    return ctx.transpose(2, 0, 1).reshape(S, H)        # [S, H]


def kernel(hidden_states, Wq, bq, Wk, bk, Wv, bv):
    hidden_states = np.ascontiguousarray(hidden_states, dtype=np.float32)
    with_bv = bool(np.any(np.asarray(bv) != 0))
    nc = _get_program(with_bv)
    in_maps = _in_maps(hidden_states, Wq, bq, Wk, bk, Wv, bv)
    last_err = None
    for _attempt in range(3):
        try:
            res = run_bass_kernel_spmd(nc, in_maps, list(range(B)))
            return np.stack(
                [_postprocess(res.results[b]["out"]) for b in range(B)], axis=0
            )
        except Exception as e:  # transient NRT device errors recover on retry
            last_err = e
            import time
            time.sleep(3)
    raise last_err


# revision 11
# speedup vs baseline: 1.1938x; 1.1938x over previous
"""ConstituencyAwareAttention Trainium2 kernel.

Strategy: pure data parallelism -- B=8 batch elements across 8 NeuronCores,
one full attention problem per core (S=1024, H=1024, nh=16, hd=64). No
collectives.

Per-core pipeline (fp16 matmul inputs, fp32 accumulation):

  * Inputs arrive pre-transposed AND pre-cast to fp16 from the host (pure
    layout/precision staging, the kernel computes in fp16 regardless), which
    halves the startup HBM load and removes every on-device cast.
  * Input DMAs are split into need-ordered pieces on three queues; X^T is
    loaded twice (once as s-quarters feeding the V projection's stationary
    operands, once as s-halves feeding the Q/K projections' moving operands)
    so the V projection starts as soon as ~1.5 MB has landed. Dummy warm-up
    matmuls keep the PE's DVFS ramp hot during the initial DMA wait.
  * All PSUM pools are double-buffered (proj x2, score x2, ctx x2 = 8 banks)
    so back-to-back accumulation groups never wait on drains.
  * Emission is software-pipelined: per pair, score matmuls (2 co-issued
    64-row matmuls per kt via tile_position) are emitted in batches of two
    kt-slots between independent filler groups (AV of the previous pair, Q/K
    projection of the next pair, deferred V-projection groups). Batching
    by two halves the number of sc<->full-K boundaries, each of which costs
    ~100ns because a tile_position LDWEIGHTS cannot overlap a full-128-row
    matmul still in flight. The score-PSUM rotation (bufs=2) tolerates
    exactly two outstanding kt slots before gating on the scalar exp.
  * The constituency fixup (undo the -0.5 penalty inside same-constituent
    [64,64] blocks) is one vector tensor_mul per (kt, half) against a
    constant [128,128] mask tile; exp itself applies the -0.5 as bias.
  * ctx is produced transposed ([65, q] per head: 64 V dims + the softmax
    denominator from a ones-column in V_aug) and DMA'd out unnormalized as
    fp16, one DMA per head. The host divides by the denominator row and
    transposes back -- no PE transposes, reciprocals or normalize multiplies
    on device.
"""

import math
import sys

if "/opt/trn_rl_repo" not in sys.path:
    sys.path.insert(0, "/opt/trn_rl_repo")

import numpy as np

import concourse.bacc as bacc
import concourse.tile as tile
from concourse import mybir
from concourse.bass_utils import run_bass_kernel_spmd

F16 = mybir.dt.float16
F32 = mybir.dt.float32

B, S, H = 8, 1024, 1024
NH, HD = 16, 64
OA = HD + 1   # per-head output rows: 64 ctx dims + denominator
P = 128
SO = S // P   # 8 S-chunks
KO = H // P   # 8 contraction chunks
PEN = 0.5
FIX = float(math.exp(PEN))
SCALE = 1.0 / math.sqrt(HD)
N_WARM = 40   # PE warm-up matmuls during the initial DMA wait

_programs = {}


def _build_program(with_bv: bool):
    nc = bacc.Bacc("TRN2", target_bir_lowering=False, debug=False)

    xt = nc.dram_tensor("xt", [H, S], F16, kind="ExternalInput").ap()
    wq = nc.dram_tensor("wq", [H, H], F16, kind="ExternalInput").ap()
    wk = nc.dram_tensor("wk", [H, H], F16, kind="ExternalInput").ap()
    wv = nc.dram_tensor("wv", [H, H], F16, kind="ExternalInput").ap()
    bq = nc.dram_tensor("bq", [H], F32, kind="ExternalInput").ap()
    bk = nc.dram_tensor("bk", [H], F32, kind="ExternalInput").ap()
    bv = nc.dram_tensor("bv", [H], F32, kind="ExternalInput").ap()
    # transposed, unnormalized output: per head 65 rows (64 ctx + denom)
    out = nc.dram_tensor("out", [NH * OA, S], F16, kind="ExternalOutput").ap()

    Exp = mybir.ActivationFunctionType.Exp

    with tile.TileContext(nc) as tc:
        with tc.tile_pool(name="persist", bufs=1) as persist:
            XTh = [persist.tile([P, KO, 512], F16, name=f"XTh{j}") for j in range(2)]
            WVh = [persist.tile([P, KO, 512], F16, name=f"WV{j}") for j in range(2)]
            WQp = [persist.tile([P, KO, 512], F16, name=f"WQ{j}") for j in range(2)]
            WKp = [persist.tile([P, KO, 512], F16, name=f"WK{j}") for j in range(2)]
            QT = [persist.tile([P, S], F16, name=f"QT{m}") for m in range(NH // 2)]
            KT = [persist.tile([P, S], F16, name=f"KT{m}") for m in range(NH // 2)]
            VA = persist.tile([P, SO, NH * OA], F16, name="VA")
            fixm = persist.tile([P, P], F16, name="fixm")
            scratch = persist.tile([P, 512], F16, name="scratch")
            nbias = persist.tile([P, 1], F32, name="nbias")
            bq_s = persist.tile([P, KO], F32, name="bq_s")
            bk_s = persist.tile([P, KO], F32, name="bk_s")

            nc.vector.memset(nbias[:], -PEN)
            # constituency fixup mask: e^PEN on the two same-block squares
            nc.vector.memset(fixm[:], 1.0)
            nc.vector.memset(fixm[0:64, 0:64], FIX)
            nc.vector.memset(fixm[64:128, 64:128], FIX)
            nc.vector.memset(scratch[:], 0.0)
            # ones-columns of V_aug (drains below only write the V parts)
            va_h = VA[:].rearrange("p s (h c) -> p s h c", c=OA)
            nc.vector.memset(va_h[:, :, :, HD : HD + 1], 1.0)

            # ---- input DMAs: three queues, need-ordered ----
            xt_r = xt.rearrange("(ho hp) s -> hp ho s", hp=P)
            wv_r = wv.rearrange("(kp_o kp) n -> kp kp_o n", kp=P)
            wq_r = wq.rearrange("(kp_o kp) n -> kp kp_o n", kp=P)
            wk_r = wk.rearrange("(kp_o kp) n -> kp kp_o n", kp=P)

            nc.sync.dma_start(out=WVh[0][:], in_=wv_r[:, :, 0:512])
            nc.gpsimd.dma_start(out=XTh[0][:], in_=xt_r[:, :, 0:512])
            nc.sync.dma_start(out=WVh[1][:], in_=wv_r[:, :, 512:1024])
            nc.gpsimd.dma_start(out=XTh[1][:], in_=xt_r[:, :, 512:1024])
            nc.scalar.dma_start(out=bq_s[:], in_=bq.rearrange("(o p) -> p o", p=P))
            nc.vector.tensor_scalar_mul(bq_s[:], bq_s[:], SCALE)
            nc.scalar.dma_start(out=bk_s[:], in_=bk.rearrange("(o p) -> p o", p=P))
            for j in range(2):
                nc.sync.dma_start(
                    out=WQp[j][:], in_=wq_r[:, :, j * 512 : (j + 1) * 512]
                )
                nc.gpsimd.dma_start(
                    out=WKp[j][:], in_=wk_r[:, :, j * 512 : (j + 1) * 512]
                )

            with (
                tc.tile_pool(name="attn", bufs=1) as attn,
                tc.tile_pool(name="proj_ps", bufs=2, space="PSUM") as proj_ps,
                tc.tile_pool(name="score_ps", bufs=2, space="PSUM") as score_ps,
                tc.tile_pool(name="ctx_ps", bufs=2, space="PSUM") as ctx_ps,
            ):
                # PE warm-up: keep the DVFS ramp hot while inputs stream in.
                for _ in range(N_WARM):
                    ps = proj_ps.tile([P, 512], F32, name="ps", tag="proj")
                    nc.tensor.matmul(
                        ps[:], fixm[:], scratch[:], start=True, stop=True
                    )

                # ---- V projection: [S,h_out] blocks into V_aug ----
                def vproj_group(so, ncol):
                    ps = proj_ps.tile([P, 512], F32, name="ps", tag="proj")
                    sq, soc = divmod(so, 4)
                    for kh in range(KO):
                        nc.tensor.matmul(
                            ps[:],
                            XTh[sq][:, kh, soc * P : (soc + 1) * P],
                            WVh[ncol][:, kh, :],
                            start=(kh == 0),
                            stop=(kh == KO - 1),
                        )
                    va_v = VA[:, so, :].rearrange("p (h c) -> p h c", c=OA)
                    nc.vector.tensor_copy(
                        va_v[:, ncol * 8 : (ncol + 1) * 8, 0:HD],
                        ps[:].rearrange("p (h c) -> p h c", c=HD),
                    )

                # heads 0-7 upfront; heads 8-15 deferred into iter 0/1 fillers
                # (with_bv needs all of VA early for the bvb add, so no defer)
                for so in range(SO):
                    vproj_group(so, 0)
                deferred_vproj = [
                    (lambda so=so: vproj_group(so, 1)) for so in range(SO)
                ]
                if with_bv:
                    for f in deferred_vproj:
                        f()
                    deferred_vproj = []
                    # out += bv exactly (softmax rows sum to 1), via a
                    # PE-broadcast of bv across partitions added to V_aug.
                    ones1 = persist.tile([1, P], F16, name="ones1")
                    nc.vector.memset(ones1[:], 1.0)
                    bv1 = persist.tile([1, H], F16, name="bv1")
                    bv1_32 = persist.tile([1, H], F32, name="bv1_32")
                    nc.sync.dma_start(out=bv1_32[:], in_=bv[None, :])
                    nc.vector.tensor_copy(bv1[:], bv1_32[:])
                    bvb = persist.tile([P, NH * OA], F16, name="bvb")
                    nc.vector.memset(bvb[:], 0.0)
                    bvb_v = bvb.rearrange("p (h c) -> p h c", c=OA)
                    for ncol in range(2):
                        psb = proj_ps.tile([P, 512], F32, name="psb", tag="proj")
                        nc.tensor.matmul(
                            psb[:], ones1[:], bv1[:, ncol * 512 : (ncol + 1) * 512],
                            start=True, stop=True,
                        )
                        nc.vector.tensor_copy(
                            bvb_v[:, ncol * 8 : (ncol + 1) * 8, 0:HD],
                            psb[:].rearrange("p (h c) -> p h c", c=HD),
                        )
                    for so in range(SO):
                        nc.vector.tensor_add(VA[:, so, :], VA[:, so, :], bvb[:])

                # ---- one Q or K projection accumulation group ----
                def qk_group(m, which, sc):
                    ps = proj_ps.tile([P, 512], F32, name="ps", tag="proj")
                    wsb = (WQp if which == 0 else WKp)[m // 4]
                    col0 = (m % 4) * P
                    for kh in range(KO):
                        nc.tensor.matmul(
                            ps[:],
                            wsb[:, kh, col0 : col0 + P],
                            XTh[sc][:, kh, :],
                            start=(kh == 0),
                            stop=(kh == KO - 1),
                        )
                    if which == 0:
                        nc.vector.tensor_scalar(
                            QT[m][:, sc * 512 : (sc + 1) * 512], ps[:],
                            SCALE, bq_s[:, m : m + 1],
                            mybir.AluOpType.mult, mybir.AluOpType.add,
                        )
                    else:
                        nc.vector.tensor_scalar_add(
                            KT[m][:, sc * 512 : (sc + 1) * 512], ps[:],
                            bk_s[:, m : m + 1],
                        )

                def qk_chunk_fillers(m):
                    return [
                        (lambda w=w, sc=sc: qk_group(m, w, sc))
                        for w in range(2)
                        for sc in range(2)
                    ]

                # prT tiles rotate 3 buffers; 2 allocated per pair
                def alloc_prT():
                    return attn.tile([P, KO, S], F16, name="prT", tag="probsT", bufs=4)

                # ---- scores + exp + fixup for one (pair, kt) ----
                def sc_slot(i, kt, prT):
                    pst = [
                        score_ps.tile([P, S], F32, name="pst", tag="score")
                        for _ in range(2)
                    ]
                    for qc in range(2):
                        for half in range(2):
                            lo = half * 64
                            nc.tensor.matmul(
                                pst[half][:, qc * 512 : (qc + 1) * 512],
                                KT[i][lo : lo + 64, kt * P : (kt + 1) * P],
                                QT[i][lo : lo + 64, qc * 512 : (qc + 1) * 512],
                                start=True,
                                stop=True,
                                tile_position=(lo, 0),
                            )
                    for half in range(2):
                        nc.scalar.activation(
                            prT[half][:, kt, :], pst[half][:], Exp, bias=nbias[:]
                        )
                        # undo the penalty inside same-constituent squares
                        nc.vector.tensor_mul(
                            prT[half][:, kt, kt * P : (kt + 1) * P],
                            prT[half][:, kt, kt * P : (kt + 1) * P],
                            fixm[:],
                        )

                # ---- AV for one (pair, half, qc): ctx^T [65, 512] ----
                def av_group(i, half, qc, prT, sb, split_dma=False):
                    h = 2 * i + half
                    ctxp = ctx_ps.tile([OA, 512], F32, name="ctxp", tag="ctx")
                    for kt in range(KO):
                        nc.tensor.matmul(
                            ctxp[:],
                            VA[:, kt, h * OA : (h + 1) * OA],
                            prT[half][:, kt, qc * 512 : (qc + 1) * 512],
                            start=(kt == 0),
                            stop=(kt == KO - 1),
                        )
                    nc.vector.tensor_copy(sb[:, qc * 512 : (qc + 1) * 512], ctxp[:])
                    if split_dma:
                        nc.gpsimd.dma_start(
                            out=out[h * OA : (h + 1) * OA,
                                    qc * 512 : (qc + 1) * 512],
                            in_=sb[:, qc * 512 : (qc + 1) * 512],
                        )
                    elif qc == 1:
                        nc.gpsimd.dma_start(
                            out=out[h * OA : (h + 1) * OA, :], in_=sb[:]
                        )

                def av_pair_fillers(i, prT, split_dma=False):
                    fills = []
                    for half in range(2):
                        sb = attn.tile([OA, S], F16, name="ctxsb", tag="ctxsb",
                                       bufs=4)
                        fills += [
                            (lambda h=half, qc=qc, pp=prT, s=sb:
                             av_group(i, h, qc, pp, s, split_dma))
                            for qc in range(2)
                        ]
                    return fills

                # ---- pipeline ----
                for f in qk_chunk_fillers(0):
                    f()

                prev_prT = None
                prT = [alloc_prT(), alloc_prT()]
                for i in range(NH // 2):
                    fillers = []
                    if prev_prT is not None:
                        fillers += av_pair_fillers(i - 1, prev_prT)
                    if i + 1 < NH // 2:
                        fillers += qk_chunk_fillers(i + 1)
                    if deferred_vproj:
                        # iter 0 tops up to 8 fillers; iter 1 takes the rest
                        # (extras are emitted after the batch loop below)
                        take = (max(0, 8 - len(fillers)) if i == 0
                                else len(deferred_vproj))
                        fillers += deferred_vproj[:take]
                        deferred_vproj = deferred_vproj[take:]
                    # interleave: one act-gated score slot, then its share
                    # of independent filler groups to hide the exp latency
                    nf = len(fillers)
                    for kt in range(KO):
                        sc_slot(i, kt, prT)
                        for fj in range(kt * nf // KO, (kt + 1) * nf // KO):
                            fillers[fj]()
                    prev_prT = prT
                    if i + 1 < NH // 2:
                        prT = [alloc_prT(), alloc_prT()]
                for f in av_pair_fillers(NH // 2 - 1, prev_prT, split_dma=True):
                    f()

    nc.compile()
    return nc


def _get_program(with_bv: bool):
    key = with_bv
    if key not in _programs:
        _programs[key] = _build_program(with_bv)
    return _programs[key]


def _in_maps(hidden_states, Wq, bq, Wk, bk, Wv, bv):
    wq = np.ascontiguousarray(Wq, np.float16)
    wk = np.ascontiguousarray(Wk, np.float16)
    wv = np.ascontiguousarray(Wv, np.float16)
    bq = np.ascontiguousarray(bq, np.float32)
    bk = np.ascontiguousarray(bk, np.float32)
    bv = np.ascontiguousarray(bv, np.float32)
    hs16 = np.asarray(hidden_states, np.float16)
    return [
        {
            "xt": np.ascontiguousarray(hs16[b].T),
            "wq": wq, "wk": wk, "wv": wv, "bq": bq, "bk": bk, "bv": bv,
        }
        for b in range(B)
    ]


def _postprocess(outT):
    # outT: [NH*OA, S] fp16, per head 64 unnormalized ctx^T rows + denom row
    r = outT.astype(np.float32).reshape(NH, OA, S)
    ctx = r[:, :HD, :] / r[:, HD : HD + 1, :]          # [NH, HD, S]
    return ctx.transpose(2, 0, 1).reshape(S, H)        # [S, H]


def kernel(hidden_states, Wq, bq, Wk, bk, Wv, bv):
    hidden_states = np.ascontiguousarray(hidden_states, dtype=np.float32)
    with_bv = bool(np.any(np.asarray(bv) != 0))
    nc = _get_program(with_bv)
    in_maps = _in_maps(hidden_states, Wq, bq, Wk, bk, Wv, bv)
    last_err = None
    for _attempt in range(3):
        try:
            res = run_bass_kernel_spmd(nc, in_maps, list(range(B)))
            return np.stack(
                [_postprocess(res.results[b]["out"]) for b in range(B)], axis=0
            )
        except Exception as e:  # transient NRT device errors recover on retry
            last_err = e
            import time
            time.sleep(3)
    raise last_err
